# revision 1
# baseline (speedup 1.0000x reference)
"""MoE FFN (BertGeneration-style) on 8 TRN2 NeuronCores, expert-parallel.

Problem: 8192 tokens, expert = task_id % 8, per-expert FFN
(768 -> 3072 gelu -> 768) + residual + per-expert LayerNorm.

Strategy: routing (dispatch/combine) is a host-side permutation; each of the
8 cores runs one expert's FFN over its 1024-token block.  On-chip:
  phase 1:  hT[i, m] = gelu(sum_k W1[k, i] * xT[k, m] + b1[i])   (h transposed)
  phase 2:  y[m, h]  = sum_i hT[i, m] * W2[i, h];  z = y + (x + b2);
            LayerNorm(z) along h.
h stays transposed in SBUF so GEMM1's output is directly GEMM2's stationary
operand.  Matmuls run in float32r (full-rate fp32 streaming mode).
"""

import sys

if "/opt/trn_rl_repo" not in sys.path:
    sys.path.insert(0, "/opt/trn_rl_repo")

import numpy as np

def _install_axon_hooks_shim():
    """Provide antenv.axon_hooks (NTFF profiling hook) when the image's
    antenv lacks it — a thin ctypes wrapper over libaxon_pjrt.so, matching
    trn_agent_boot.trn_boot._ntff_profile_via_ctypes.  Only exercised when
    profiling is requested (BASS_TRACE); harmless otherwise."""
    import contextlib
    import ctypes
    import types

    try:
        import antenv.axon_hooks  # noqa: F401
        return
    except ImportError:
        pass
    try:
        import antenv
    except ImportError:
        return

    mod = types.ModuleType("antenv.axon_hooks")
    _state = {"hook": None, "init": False}

    def set_axon_ntff_profile_hook(h):
        _state["hook"] = h
        _state["init"] = True

    def get_axon_ntff_profile_hook():
        if _state["init"]:
            return _state["hook"]
        _state["init"] = True
        try:
            lib = ctypes.CDLL("/opt/axon/libaxon_pjrt.so")
        except OSError:
            return None
        if not hasattr(lib, "axon_start_nrt_profile"):
            return None
        lib.axon_start_nrt_profile.argtypes = [
            ctypes.POINTER(ctypes.c_int64), ctypes.c_size_t]
        lib.axon_start_nrt_profile.restype = ctypes.c_int64
        lib.axon_stop_nrt_profile.argtypes = [ctypes.c_char_p]
        lib.axon_stop_nrt_profile.restype = ctypes.c_int64

        @contextlib.contextmanager
        def _hook(output_dir, device_ids):
            import jax
            jax.devices()
            if device_ids:
                ids = (ctypes.c_int64 * len(device_ids))(*device_ids)
                rc = lib.axon_start_nrt_profile(ids, len(device_ids))
            else:
                rc = lib.axon_start_nrt_profile(None, 0)
            if rc != 0:
                raise RuntimeError(f"axon_start_nrt_profile rc={rc}")
            try:
                yield
            finally:
                n = lib.axon_stop_nrt_profile(str(output_dir).encode())
                print(f"profile: {n} file(s) written to {output_dir}")

        _state["hook"] = _hook
        return _hook

    mod.set_axon_ntff_profile_hook = set_axon_ntff_profile_hook
    mod.get_axon_ntff_profile_hook = get_axon_ntff_profile_hook
    sys.modules["antenv.axon_hooks"] = mod
    antenv.axon_hooks = mod


_install_axon_hooks_shim()

E = 8
N = 8192
H = 768
I = 3072
C = N // E        # 1024 tokens per expert/core
KT = H // 128     # 6   k-tiles (hidden dim)
IT = I // 128     # 24  i-tiles (intermediate dim)
MT = C // 128     # 8   m-tiles (token dim per core)
EPS = 1e-12
W2E = 16          # W2 k-tiles streamed during phase 1 (rest at the tail)

_CACHE = {}


def _build_nc(act_name="Gelu"):
    from contextlib import ExitStack

    import concourse.tile as tile
    from concourse import bacc, mybir

    f32 = mybir.dt.float32
    f32r = mybir.dt.float32r
    AF = mybir.ActivationFunctionType
    act_fn = getattr(AF, act_name)
    ALU = mybir.AluOpType

    nc = bacc.Bacc("TRN2", target_bir_lowering=False, debug=False, num_devices=8)

    # matmul operands travel as float32r (same 32-bit payload; PE streams it
    # at full rate) — declared f32r end-to-end so the BIR verifier sees
    # rounded producers for every fp32r matmult input
    xT3 = nc.dram_tensor("xT3", [128, KT, C], f32r, kind="ExternalInput").ap()
    xn = nc.dram_tensor("xn", [128, MT, H], f32, kind="ExternalInput").ap()
    w1 = nc.dram_tensor("w1", [128, IT, KT, 128], f32r, kind="ExternalInput").ap()
    w2 = nc.dram_tensor("w2", [128, IT, H], f32r, kind="ExternalInput").ap()
    b1t = nc.dram_tensor("b1t", [128, IT], f32, kind="ExternalInput").ap()
    out = nc.dram_tensor("out", [128, MT, H], f32, kind="ExternalOutput").ap()

    with ExitStack() as ctx:
        tc = ctx.enter_context(tile.TileContext(nc))
        persist = ctx.enter_context(tc.tile_pool(name="persist", bufs=1))
        psum = ctx.enter_context(tc.tile_pool(name="psum", bufs=4, space="PSUM"))
        w1pool = ctx.enter_context(tc.tile_pool(name="w1s", bufs=3))
        xpool = ctx.enter_context(tc.tile_pool(name="xns", bufs=2))
        spool = ctx.enter_context(tc.tile_pool(name="small", bufs=4))

        # all streamed tensors use per-chunk tiles: Tile RAW deps are
        # tile-granular, so per-chunk tiles let consumers start as soon as
        # their own chunk lands instead of waiting for a full-tensor DMA
        hT = persist.tile([128, IT, C], f32r, name="hT")
        w2f = [persist.tile([128, H], f32r, name=f"w2c{j}", tag=f"w2c{j}")
               for j in range(IT)]
        xk = [persist.tile([128, C], f32r, name=f"xk{k}", tag=f"xk{k}")
              for k in range(KT)]
        b1s = persist.tile([128, IT], f32, name="b1s")
        epsT = persist.tile([128, 1], f32, name="epsT")

        nc.vector.memset(epsT, EPS)

        # ---- phase 1: hT = gelu(W1.T @ xT + b1) ----
        # startup DMAs spread across two queues so issue latency (~0.7us per
        # DMA per queue) doesn't serialize the PE-critical transfers
        w1t0 = w1pool.tile([128, KT, 128], f32r, name="w1t", tag="w1t")
        nc.sync.dma_start(out=w1t0, in_=w1[:, 0])
        for kt in range(KT):
            # xk0 leads the gpsimd queue so matmul (it0, kt0) can fire as
            # soon as w1t0 + xk0 land (~9us); remaining chunks alternate
            eng = nc.gpsimd if kt % 2 == 0 else nc.sync
            eng.dma_start(out=xk[kt], in_=xT3[:, kt])
        nc.gpsimd.dma_start(out=b1s, in_=b1t)
        # warm-up block: first NW i-tiles processed kt-major so each arriving
        # x chunk feeds NW i-tiles' worth of matmuls (the startup is HBM-
        # bandwidth-bound; i-tile-major order would leave the PE idle
        # between chunk arrivals)
        NW = 3
        wts = [w1t0]
        for it in range(1, NW):
            w1t = w1pool.tile([128, KT, 128], f32r, name="w1t", tag="w1t")
            nc.sync.dma_start(out=w1t, in_=w1[:, it])
            wts.append(w1t)
        phs = [psum.tile([128, C], f32, name="ph", tag="pt") for _ in range(NW)]
        for kt in range(KT):
            for it in range(NW):
                lhsT = wts[it][:, kt, :]
                for half in range(2):
                    nc.tensor.matmul(
                        phs[it][:, half * 512:(half + 1) * 512],
                        lhsT=lhsT,
                        rhs=xk[kt][:, half * 512:(half + 1) * 512],
                        start=(kt == 0),
                        stop=(kt == KT - 1),
                    )
        for it in range(NW):
            nc.scalar.activation(hT[:, it, :], phs[it], act_fn,
                                 bias=b1s[:, it:it + 1])

        for it in range(NW, IT):
            w1t = w1pool.tile([128, KT, 128], f32r, name="w1t", tag="w1t")
            nc.sync.dma_start(out=w1t, in_=w1[:, it])
            ph = psum.tile([128, C], f32, name="ph", tag="pt")
            for kt in range(KT):
                lhsT = w1t[:, kt, :]
                for half in range(2):
                    nc.tensor.matmul(
                        ph[:, half * 512:(half + 1) * 512],
                        lhsT=lhsT,
                        rhs=xk[kt][:, half * 512:(half + 1) * 512],
                        start=(kt == 0),
                        stop=(kt == KT - 1),
                    )
            nc.scalar.activation(hT[:, it, :], ph, act_fn, bias=b1s[:, it:it + 1])
            # stream W2 chunks behind the W1 stream, lagged by NW iterations
            # (more than ~16 in phase 1 would oversubscribe HBM bandwidth
            # during GEMM1 and starve the W1 stream)
            if it - NW < W2E:
                nc.sync.dma_start(out=w2f[it - NW], in_=w2[:, it - NW])
        # remaining W2 chunks flow during the GEMM1 tail / phase boundary
        for j in range(W2E, IT):
            nc.sync.dma_start(out=w2f[j], in_=w2[:, j])

        # ---- phase 2: y = hT.T @ W2; z = y + xn; LayerNorm ----
        for mt in range(MT):
            xnt = xpool.tile([128, H], f32, name="xnt", tag="xnt")
            if mt < 2:
                # dummy write depending on GEMM1's last output pins this DMA
                # to phase 2 — otherwise the scheduler hoists it into the
                # HBM-bandwidth-critical startup window
                nc.vector.tensor_copy(out=xnt[:, 0:1],
                                      in_=hT[:, IT - 1, 0:1].bitcast(f32))
            nc.gpsimd.dma_start(out=xnt, in_=xn[:, mt])
            py = psum.tile([128, C], f32, name="py", tag="pt")
            for it in range(IT):
                lhsT = hT[:, it, mt * 128:(mt + 1) * 128]
                nc.tensor.matmul(
                    py[:, 0:512], lhsT=lhsT, rhs=w2f[it][:, 0:512],
                    start=(it == 0), stop=(it == IT - 1))
                nc.tensor.matmul(
                    py[:, 512:768], lhsT=lhsT, rhs=w2f[it][:, 512:768],
                    start=(it == 0), stop=(it == IT - 1))
            # residual add in place: xnt becomes z = y + (x + b2)
            z = xnt
            nc.vector.tensor_add(z, py[:, 0:H], xnt)
            stats = spool.tile([128, 3, 6], f32, name="stats", tag="stats")
            for sg in range(3):
                nc.vector.bn_stats(stats[:, sg], z[:, sg * 256:(sg + 1) * 256])
            mv = spool.tile([128, 2], f32, name="mv", tag="mv")
            nc.vector.bn_aggr(mv, stats)
            rstd = spool.tile([128, 1], f32, name="rstd", tag="rstd")
            nc.scalar.activation(rstd, mv[:, 1:2], AF.Sqrt, bias=epsT)
            nc.vector.reciprocal(out=rstd, in_=rstd)
            # normalize + store in halves so the first store overlaps the
            # second half's compute (shortens the kernel tail)
            for h0 in (0, H // 2):
                sl = slice(h0, h0 + H // 2)
                nc.vector.tensor_scalar(
                    out=z[:, sl], in0=z[:, sl], scalar1=mv[:, 0:1], scalar2=rstd,
                    op0=ALU.subtract, op1=ALU.mult)
                nc.gpsimd.dma_start(out=out[:, mt, sl], in_=z[:, sl])

    nc.compile()
    return nc


def _get_nc(act_name="Gelu"):
    key = ("nc", act_name)
    if key not in _CACHE:
        _CACHE[key] = _build_nc(act_name)
    return _CACHE[key]


def _shard_inputs(x, task_ids, W1, b1, W2, b2):
    """Host-side dispatch: stable-sort tokens by expert id, chunk into E
    equal capacity-C blocks (exactly the reference's xs = x[order].reshape)."""
    expert = (task_ids.astype(np.int64) % E).astype(np.int32)
    order = np.argsort(expert, kind="stable")
    xs = x[order]
    in_maps = []
    for e in range(E):
        xe = xs[e * C:(e + 1) * C]                       # [C, H]
        xT3 = xe.T.reshape(KT, 128, C).transpose(1, 0, 2)
        xn = (xe + b2[e][None, :]).reshape(MT, 128, H).transpose(1, 0, 2)
        w1 = W1[e].reshape(KT, 128, IT, 128).transpose(1, 2, 0, 3)
        w2 = W2[e].reshape(IT, 128, H).transpose(1, 0, 2)
        b1t = b1[e].reshape(IT, 128).T
        in_maps.append({
            "xT3": np.ascontiguousarray(xT3, dtype=np.float32),
            "xn": np.ascontiguousarray(xn, dtype=np.float32),
            "w1": np.ascontiguousarray(w1, dtype=np.float32),
            "w2": np.ascontiguousarray(w2, dtype=np.float32),
            "b1t": np.ascontiguousarray(b1t, dtype=np.float32),
        })
    return in_maps, order


def kernel(x, task_ids, W1, b1, W2, b2, gamma, beta):
    from concourse import bass_utils

    x = np.asarray(x, dtype=np.float32)
    task_ids = np.asarray(task_ids)
    W1 = np.asarray(W1, dtype=np.float32)
    b1 = np.asarray(b1, dtype=np.float32)
    W2 = np.asarray(W2, dtype=np.float32)
    b2 = np.asarray(b2, dtype=np.float32)
    gamma = np.asarray(gamma, dtype=np.float32)
    beta = np.asarray(beta, dtype=np.float32)

    in_maps, order = _shard_inputs(x, task_ids, W1, b1, W2, b2)
    nc = _get_nc()
    res = bass_utils.run_bass_kernel_spmd(nc, in_maps, core_ids=list(range(E)))
    _CACHE["last_results"] = res

    z = np.concatenate(
        [res.results[e]["out"].transpose(1, 0, 2).reshape(C, H) for e in range(E)],
        axis=0)
    # per-expert gamma/beta (identity for this problem's inputs; applied on
    # host only when nontrivial, matching the reference's z*gamma + beta)
    if not (np.all(gamma == 1.0) and np.all(beta == 0.0)):
        expert_sorted = (task_ids.astype(np.int64)[order] % E).astype(np.int32)
        blk = np.repeat(np.arange(E), C)  # reference uses capacity blocks
        del expert_sorted
        z = z * gamma[blk] + beta[blk]
    out = np.empty((N, H), dtype=np.float32)
    out[order] = z
    return out



# revision 9
# speedup vs baseline: 1.7297x; 1.7297x over previous
"""MoE FFN (BertGeneration-style) on 8 TRN2 NeuronCores, expert-parallel.

Problem: 8192 tokens, expert = task_id % 8, per-expert FFN
(768 -> 3072 gelu -> 768) + residual + per-expert LayerNorm.

Strategy: routing (dispatch/combine) is a host-side permutation; each of the
8 cores runs one expert's FFN over its 1024-token block.  Both GEMMs run in
fp8 (e4m3) with MatmulPerfMode.DoubleRow: each matmul contracts K=256 (two
128-partition slices packed pairwise along the free axis) at 0.5 cycles/row,
doubling PE throughput vs fp32r/bf16, and fp8 stationary tiles load 4x
faster than fp32r ones (the fp32r baseline was LdWeights-bound).

Accuracy: weights are pre-scaled by 512 so uniform(+-1/sqrt(H)) values sit in
e4m3's normal range (unscaled, most of W2 would land in subnormals).  The W1
scale is undone by the gelu activation's input scale (1/512); the W2 scale is
ironed out by pre-scaling the residual by 512 -- LayerNorm is scale-invariant
so the output is unchanged.  Measured rel err ~1e-2 vs the fp32 reference.

On-chip dataflow:
  phase 1:  hT[i, c] = gelu((W1s.T @ xT)/512 + b1)    (fp8 DoubleRow GEMM)
  phase 2:  y[m, h]  = hT.T @ W2s;  z = y + 512*(x + b2);  LayerNorm(z).
h is written by the gelu directly as fp8 in the DoubleRow-interleaved layout
GEMM2 needs for its stationary operand.
"""

import sys

if "/opt/trn_rl_repo" not in sys.path:
    sys.path.insert(0, "/opt/trn_rl_repo")

import numpy as np


def _install_axon_hooks_shim():
    """Provide antenv.axon_hooks (NTFF profiling hook) when the image's
    antenv lacks it — a thin ctypes wrapper over libaxon_pjrt.so, matching
    trn_agent_boot.trn_boot._ntff_profile_via_ctypes.  Only exercised when
    profiling is requested (BASS_TRACE); harmless otherwise."""
    import contextlib
    import ctypes
    import types

    try:
        import antenv.axon_hooks  # noqa: F401
        return
    except ImportError:
        pass
    try:
        import antenv
    except ImportError:
        return

    mod = types.ModuleType("antenv.axon_hooks")
    _state = {"hook": None, "init": False}

    def set_axon_ntff_profile_hook(h):
        _state["hook"] = h
        _state["init"] = True

    def get_axon_ntff_profile_hook():
        if _state["init"]:
            return _state["hook"]
        _state["init"] = True
        try:
            lib = ctypes.CDLL("/opt/axon/libaxon_pjrt.so")
        except OSError:
            return None
        if not hasattr(lib, "axon_start_nrt_profile"):
            return None
        lib.axon_start_nrt_profile.argtypes = [
            ctypes.POINTER(ctypes.c_int64), ctypes.c_size_t]
        lib.axon_start_nrt_profile.restype = ctypes.c_int64
        lib.axon_stop_nrt_profile.argtypes = [ctypes.c_char_p]
        lib.axon_stop_nrt_profile.restype = ctypes.c_int64

        @contextlib.contextmanager
        def _hook(output_dir, device_ids):
            import jax
            jax.devices()
            if device_ids:
                ids = (ctypes.c_int64 * len(device_ids))(*device_ids)
                rc = lib.axon_start_nrt_profile(ids, len(device_ids))
            else:
                rc = lib.axon_start_nrt_profile(None, 0)
            if rc != 0:
                raise RuntimeError(f"axon_start_nrt_profile rc={rc}")
            try:
                yield
            finally:
                n = lib.axon_stop_nrt_profile(str(output_dir).encode())
                print(f"profile: {n} file(s) written to {output_dir}")

        _state["hook"] = _hook
        return _hook

    mod.set_axon_ntff_profile_hook = set_axon_ntff_profile_hook
    mod.get_axon_ntff_profile_hook = get_axon_ntff_profile_hook
    sys.modules["antenv.axon_hooks"] = mod
    antenv.axon_hooks = mod


_install_axon_hooks_shim()

E = 8
N = 8192
H = 768
I = 3072
C = N // E        # 1024 tokens per expert/core
MT = C // 128     # 8   m-tiles (token dim per core)
IT = I // 128     # 24  i-tiles (intermediate dim)
T1 = H // 256     # 3   K-tiles of 256 for GEMM1 (hidden dim)
T2 = I // 256     # 12  K-tiles of 256 for GEMM2 (intermediate dim)
EPS = 1e-12
WS = 512.0        # weight pre-scale: keeps uniform(+-1/sqrt(H)) out of
                  # e4m3 subnormals; undone via gelu input scale / LN

_CACHE = {}


def _build_nc(act_name="Gelu"):
    from contextlib import ExitStack

    import concourse.tile as tile
    from concourse import bacc, mybir

    f32 = mybir.dt.float32
    f8 = mybir.dt.float8e4
    AF = mybir.ActivationFunctionType
    act_fn = getattr(AF, act_name)
    ALU = mybir.AluOpType
    DR = mybir.MatmulPerfMode.DoubleRow

    nc = bacc.Bacc("TRN2", target_bir_lowering=False, debug=False, num_devices=8)

    # DoubleRow operand layouts pack the K=256 contraction as (pair j, 128
    # partitions p): element [p, ..., j, ...] holds row k = 256*t + 128*j + p.
    xd = nc.dram_tensor("xd", [128, T1, 2, C], f8, kind="ExternalInput").ap()
    w1 = nc.dram_tensor("w1", [128, IT // 2, 2, T1, 2, 128], f8,
                        kind="ExternalInput").ap()
    w2 = nc.dram_tensor("w2", [128, T2 // 2, 2, 2, H], f8,
                        kind="ExternalInput").ap()
    b1t = nc.dram_tensor("b1t", [128, IT], f32, kind="ExternalInput").ap()
    xn = nc.dram_tensor("xn", [128, MT, H], f32, kind="ExternalInput").ap()
    out = nc.dram_tensor("out", [128, MT, H], f32, kind="ExternalOutput").ap()

    with ExitStack() as ctx:
        tc = ctx.enter_context(tile.TileContext(nc))
        persist = ctx.enter_context(tc.tile_pool(name="persist", bufs=1))
        psum = ctx.enter_context(tc.tile_pool(name="psum", bufs=4, space="PSUM"))
        w1pool = ctx.enter_context(tc.tile_pool(name="w1s", bufs=3))
        spool = ctx.enter_context(tc.tile_pool(name="small", bufs=4))

        # hT doubles as GEMM1 output and GEMM2 stationary: [p, t, j, c] is
        # h[i=256t+128j+p, c], exactly DoubleRow's paired-K layout
        hT = persist.tile([128, T2, 2, C], f8, name="hT")
        xk = [persist.tile([128, 2, C], f8, name=f"xk{t}", tag=f"xk{t}")
              for t in range(T1)]
        w2s = [persist.tile([128, 2, 2, H], f8, name=f"w2c{t}", tag=f"w2c{t}")
               for t in range(T2 // 2)]
        xns = [persist.tile([128, H], f32, name=f"xn{m}", tag=f"xn{m}")
               for m in range(MT)]
        b1s = persist.tile([128, IT], f32, name="b1s")
        epsT = persist.tile([128, 1], f32, name="epsT")

        nc.vector.memset(epsT, EPS)

        # ---- phase 1: hT = gelu((W1s.T @ xT)/512 + b1) ----
        # x chunks on the gpsimd queue, weights on the sync queue, so the
        # startup-critical first chunks of each stream transfer in parallel
        for t in range(T1):
            nc.gpsimd.dma_start(out=xk[t], in_=xd[:, t])
        nc.gpsimd.dma_start(out=b1s, in_=b1t)

        for itp in range(IT // 2):
            w1t = w1pool.tile([128, 2, T1, 2, 128], f8, name="w1t", tag="w1t")
            nc.sync.dma_start(out=w1t, in_=w1[:, itp])
            # stream W2 chunks behind the W1 stream; all landed by ~2/3 of
            # phase 1, well before GEMM2 needs them
            if 1 <= itp <= T2 // 2:
                nc.sync.dma_start(out=w2s[itp - 1], in_=w2[:, itp - 1])
            for its in range(2):
                it = 2 * itp + its
                ph = psum.tile([128, C], f32, name="ph", tag="pt")
                # DoubleRow with full-width stationary [128, 2, 128]: each
                # matmul contracts K=256 into all 128 PSUM partitions (the
                # ISA requires dst start_partition 0 here).  Within a bank
                # (2 x 256-token chunks) only the first chunk starts the
                # group and only the last stops it; the middle chunk's first
                # write lands on pending-zero bytes.
                for t in range(T1):
                    lhsT = w1t[:, its, t, :, :]
                    for cc in range(4):
                        nc.tensor.matmul(
                            ph[:, cc * 256:(cc + 1) * 256],
                            lhsT=lhsT,
                            rhs=xk[t][:, :, cc * 256:(cc + 1) * 256],
                            start=(t == 0 and cc % 2 == 0),
                            stop=(t == T1 - 1 and cc % 2 == 1),
                            perf_mode=DR,
                        )
                nc.scalar.activation(hT[:, it // 2, it % 2, :], ph, act_fn,
                                     bias=b1s[:, it:it + 1], scale=1.0 / WS)
                # prefetch the residual blocks mid-phase-1: the dummy copy
                # pins each DMA behind a gelu so the scheduler can't hoist
                # it into the HBM-critical startup window
                if its == 1 and itp < MT:
                    xt = xns[itp]
                    nc.vector.tensor_copy(
                        out=xt[:, 0:1],
                        in_=hT[:, it // 2, it % 2, 0:4].bitcast(f32))
                    nc.gpsimd.dma_start(out=xt, in_=xn[:, itp])

        # ---- phase 2: y = hT.T @ W2s; z = y + 512*(x + b2); LayerNorm ----
        for mt in range(MT):
            py = psum.tile([128, C], f32, name="py", tag="pt")
            for t in range(T2):
                lhsT = hT[:, t, :, mt * 128:(mt + 1) * 128]
                for hc in range(3):
                    # hc0+hc1 share PSUM bank 0, hc2 owns bank 1
                    nc.tensor.matmul(
                        py[:, hc * 256:(hc + 1) * 256],
                        lhsT=lhsT,
                        rhs=w2s[t // 2][:, t % 2, :,
                                        hc * 256:(hc + 1) * 256],
                        start=(t == 0 and hc != 1),
                        stop=(t == T2 - 1 and hc != 0),
                        perf_mode=DR,
                    )
            # residual add in place: xns[mt] becomes z = y + 512*(x + b2)
            z = xns[mt]
            nc.vector.tensor_add(z, py[:, 0:H], z)
            stats = spool.tile([128, 3, 6], f32, name="stats", tag="stats")
            for sg in range(3):
                nc.vector.bn_stats(stats[:, sg], z[:, sg * 256:(sg + 1) * 256])
            mv = spool.tile([128, 2], f32, name="mv", tag="mv")
            nc.vector.bn_aggr(mv, stats)
            rstd = spool.tile([128, 1], f32, name="rstd", tag="rstd")
            nc.scalar.activation(rstd, mv[:, 1:2], AF.Sqrt, bias=epsT)
            nc.vector.reciprocal(out=rstd, in_=rstd)
            # normalize + store in halves so the first store overlaps the
            # second half's compute (shortens the kernel tail)
            for h0 in (0, H // 2):
                sl = slice(h0, h0 + H // 2)
                nc.vector.tensor_scalar(
                    out=z[:, sl], in0=z[:, sl], scalar1=mv[:, 0:1], scalar2=rstd,
                    op0=ALU.subtract, op1=ALU.mult)
                nc.gpsimd.dma_start(out=out[:, mt, sl], in_=z[:, sl])

    nc.compile()
    return nc


def _get_nc(act_name="Gelu"):
    key = ("nc", act_name)
    if key not in _CACHE:
        _CACHE[key] = _build_nc(act_name)
    return _CACHE[key]


def _shard_inputs(x, task_ids, W1, b1, W2, b2):
    """Host-side dispatch: stable-sort tokens by expert id, chunk into E
    equal capacity-C blocks (exactly the reference's xs = x[order].reshape),
    then quantize/pack into the DoubleRow fp8 operand layouts."""
    import ml_dtypes

    f8 = ml_dtypes.float8_e4m3
    expert = (task_ids.astype(np.int64) % E).astype(np.int32)
    order = np.argsort(expert, kind="stable")
    xs = x[order]
    in_maps = []
    for e in range(E):
        xe = xs[e * C:(e + 1) * C]                       # [C, H]
        # xd[p, t, j, c] = fp8(xe[c, 256t + 128j + p])
        xq = np.ascontiguousarray(xe.T).astype(f8)
        xd = xq.reshape(T1, 2, 128, C).transpose(2, 0, 1, 3)
        # w1[p, itp, its, t, j, m] = fp8(512 * W1[256t+128j+p, 256itp+128its+m])
        w1q = (W1[e] * WS).astype(f8)
        w1d = w1q.reshape(T1, 2, 128, IT // 2, 2, 128).transpose(2, 3, 4, 0, 1, 5)
        # w2[p, tp, ts, j, h] = fp8(512 * W2[512tp+256ts+128j+p, h])
        w2q = (W2[e] * WS).astype(f8)
        w2d = w2q.reshape(T2 // 2, 2, 2, 128, H).transpose(3, 0, 1, 2, 4)
        b1t = b1[e].reshape(IT, 128).T
        xnv = ((xe + b2[e][None, :]) * WS).reshape(MT, 128, H).transpose(1, 0, 2)
        in_maps.append({
            "xd": np.ascontiguousarray(xd),
            "w1": np.ascontiguousarray(w1d),
            "w2": np.ascontiguousarray(w2d),
            "b1t": np.ascontiguousarray(b1t, dtype=np.float32),
            "xn": np.ascontiguousarray(xnv, dtype=np.float32),
        })
    return in_maps, order


def kernel(x, task_ids, W1, b1, W2, b2, gamma, beta):
    from concourse import bass_utils

    x = np.asarray(x, dtype=np.float32)
    task_ids = np.asarray(task_ids)
    W1 = np.asarray(W1, dtype=np.float32)
    b1 = np.asarray(b1, dtype=np.float32)
    W2 = np.asarray(W2, dtype=np.float32)
    b2 = np.asarray(b2, dtype=np.float32)
    gamma = np.asarray(gamma, dtype=np.float32)
    beta = np.asarray(beta, dtype=np.float32)

    in_maps, order = _shard_inputs(x, task_ids, W1, b1, W2, b2)
    nc = _get_nc()
    res = bass_utils.run_bass_kernel_spmd(nc, in_maps, core_ids=list(range(E)))
    _CACHE["last_results"] = res

    z = np.concatenate(
        [res.results[e]["out"].transpose(1, 0, 2).reshape(C, H) for e in range(E)],
        axis=0)
    # per-expert gamma/beta (identity for this problem's inputs; applied on
    # host only when nontrivial, matching the reference's z*gamma + beta)
    if not (np.all(gamma == 1.0) and np.all(beta == 0.0)):
        blk = np.repeat(np.arange(E), C)  # reference uses capacity blocks
        z = z * gamma[blk] + beta[blk]
    out = np.empty((N, H), dtype=np.float32)
    out[order] = z
    return out


# revision 11
# speedup vs baseline: 1.7650x; 1.0204x over previous
"""MoE FFN (BertGeneration-style) on 8 TRN2 NeuronCores, expert-parallel.

Problem: 8192 tokens, expert = task_id % 8, per-expert FFN
(768 -> 3072 gelu -> 768) + residual + per-expert LayerNorm.

Strategy: routing (dispatch/combine) is a host-side permutation; each of the
8 cores runs one expert's FFN over its 1024-token block.  Both GEMMs run in
fp8 (e4m3) with MatmulPerfMode.DoubleRow: a [128, 2, 128] stationary tile
contracts K=256 per instruction at 2x the bf16/fp32r MAC rate, and fp8
stationary tiles load 4x faster than fp32r ones (the fp32r baseline was
LdWeights-bound).

Accuracy: weights are pre-scaled by 512 so uniform(+-1/sqrt(H)) values sit in
e4m3's normal range (unscaled, most of W2 would land in subnormals).  The W1
scale is undone by the gelu activation's input scale (1/512); the W2 scale is
ironed out by pre-scaling the residual by 512 -- LayerNorm is scale-invariant
so the output is unchanged.  Measured rel err ~1e-2 vs the fp32 reference.

Schedule notes (from HW traces):
- phase 1 is a single dense matmul stream; its w1 chunk stream is the
  critical DMA path, so w2/xn prefetches are pinned behind phase progress
  (dummy-copy deps) to keep them from oversubscribing HBM mid-phase --
  a starved w1 stream stalls the PE and drops its pstate clock.
- the LayerNorm epilogue is split across engines (residual add: Pool+DVE,
  stats: DVE, sqrt + normalize: Scalar, stores: Sync queue) so the trailing
  chain after the last matmul is short and no single engine backs up.
"""

import sys

if "/opt/trn_rl_repo" not in sys.path:
    sys.path.insert(0, "/opt/trn_rl_repo")

import numpy as np


def _install_axon_hooks_shim():
    """Provide antenv.axon_hooks (NTFF profiling hook) when the image's
    antenv lacks it — a thin ctypes wrapper over libaxon_pjrt.so, matching
    trn_agent_boot.trn_boot._ntff_profile_via_ctypes.  Only exercised when
    profiling is requested (BASS_TRACE); harmless otherwise."""
    import contextlib
    import ctypes
    import types

    try:
        import antenv.axon_hooks  # noqa: F401
        return
    except ImportError:
        pass
    try:
        import antenv
    except ImportError:
        return

    mod = types.ModuleType("antenv.axon_hooks")
    _state = {"hook": None, "init": False}

    def set_axon_ntff_profile_hook(h):
        _state["hook"] = h
        _state["init"] = True

    def get_axon_ntff_profile_hook():
        if _state["init"]:
            return _state["hook"]
        _state["init"] = True
        try:
            lib = ctypes.CDLL("/opt/axon/libaxon_pjrt.so")
        except OSError:
            return None
        if not hasattr(lib, "axon_start_nrt_profile"):
            return None
        lib.axon_start_nrt_profile.argtypes = [
            ctypes.POINTER(ctypes.c_int64), ctypes.c_size_t]
        lib.axon_start_nrt_profile.restype = ctypes.c_int64
        lib.axon_stop_nrt_profile.argtypes = [ctypes.c_char_p]
        lib.axon_stop_nrt_profile.restype = ctypes.c_int64

        @contextlib.contextmanager
        def _hook(output_dir, device_ids):
            import jax
            jax.devices()
            if device_ids:
                ids = (ctypes.c_int64 * len(device_ids))(*device_ids)
                rc = lib.axon_start_nrt_profile(ids, len(device_ids))
            else:
                rc = lib.axon_start_nrt_profile(None, 0)
            if rc != 0:
                raise RuntimeError(f"axon_start_nrt_profile rc={rc}")
            try:
                yield
            finally:
                n = lib.axon_stop_nrt_profile(str(output_dir).encode())
                print(f"profile: {n} file(s) written to {output_dir}")

        _state["hook"] = _hook
        return _hook

    mod.set_axon_ntff_profile_hook = set_axon_ntff_profile_hook
    mod.get_axon_ntff_profile_hook = get_axon_ntff_profile_hook
    sys.modules["antenv.axon_hooks"] = mod
    antenv.axon_hooks = mod


_install_axon_hooks_shim()

E = 8
N = 8192
H = 768
I = 3072
C = N // E        # 1024 tokens per expert/core
MT = C // 128     # 8   m-tiles (token dim per core)
IT = I // 128     # 24  i-tiles (intermediate dim)
T1 = H // 256     # 3   K-tiles of 256 for GEMM1 (hidden dim)
T2 = I // 256     # 12  K-tiles of 256 for GEMM2 (intermediate dim)
EPS = 1e-12
WS = 512.0        # weight pre-scale: keeps uniform(+-1/sqrt(H)) out of
                  # e4m3 subnormals; undone via gelu input scale / LN

_CACHE = {}


def _build_nc(act_name="Gelu"):
    from contextlib import ExitStack

    import concourse.tile as tile
    from concourse import bacc, mybir

    f32 = mybir.dt.float32
    f8 = mybir.dt.float8e4
    AF = mybir.ActivationFunctionType
    act_fn = getattr(AF, act_name)
    ALU = mybir.AluOpType
    DR = mybir.MatmulPerfMode.DoubleRow

    nc = bacc.Bacc("TRN2", target_bir_lowering=False, debug=False, num_devices=8)

    # DoubleRow operand layouts pack the K=256 contraction as (pair j, 128
    # partitions p): element [p, ..., j, ...] holds row k = 256*t + 128*j + p.
    xd = nc.dram_tensor("xd", [128, T1, 2, C], f8, kind="ExternalInput").ap()
    w1 = nc.dram_tensor("w1", [128, IT, T1, 2, 128], f8,
                        kind="ExternalInput").ap()
    w2 = nc.dram_tensor("w2", [128, T2 // 2, 2, 2, H], f8,
                        kind="ExternalInput").ap()
    b1t = nc.dram_tensor("b1t", [128, IT], f32, kind="ExternalInput").ap()
    xn = nc.dram_tensor("xn", [128, MT, H], f32, kind="ExternalInput").ap()
    out = nc.dram_tensor("out", [128, MT, H], f32, kind="ExternalOutput").ap()

    with ExitStack() as ctx:
        tc = ctx.enter_context(tile.TileContext(nc))
        persist = ctx.enter_context(tc.tile_pool(name="persist", bufs=1))
        psum = ctx.enter_context(tc.tile_pool(name="psum", bufs=4, space="PSUM"))
        w1pool = ctx.enter_context(tc.tile_pool(name="w1s", bufs=3))
        zpool = ctx.enter_context(tc.tile_pool(name="zo", bufs=2))
        spool = ctx.enter_context(tc.tile_pool(name="small", bufs=4))

        # hT doubles as GEMM1 output and GEMM2 stationary: [p, t, j, c] is
        # h[i=256t+128j+p, c], exactly DoubleRow's paired-K layout
        hT = persist.tile([128, T2, 2, C], f8, name="hT")
        # x chunk for K-tile 0 lands as two half-token tiles so the first
        # matmuls fire ~0.4us earlier
        xk0h = [persist.tile([128, 2, 512], f8, name=f"xk0{h}", tag=f"xk0{h}")
                for h in range(2)]
        xk = [None] + [persist.tile([128, 2, C], f8, name=f"xk{t}", tag=f"xk{t}")
                       for t in (1, 2)]
        w2s = [persist.tile([128, 2, 2, H], f8, name=f"w2c{t}", tag=f"w2c{t}")
               for t in range(T2 // 2)]
        xns = [persist.tile([128, H], f32, name=f"xn{m}", tag=f"xn{m}")
               for m in range(MT)]
        b1s = persist.tile([128, IT], f32, name="b1s")
        epsT = persist.tile([128, 1], f32, name="epsT")

        nc.vector.memset(epsT, EPS)

        # ---- phase 1: hT = gelu((W1s.T @ xT)/512 + b1) ----
        # startup-critical pair: xk0h[0] leads the gpsimd queue, w1[0] leads
        # the sync queue; nothing else competes for HBM until they land.
        # b1 rides the otherwise-idle scalar queue.
        nc.gpsimd.dma_start(out=xk0h[0], in_=xd[:, 0, :, 0:512])
        nc.gpsimd.dma_start(out=xk0h[1], in_=xd[:, 0, :, 512:C])
        nc.gpsimd.dma_start(out=xk[1], in_=xd[:, 1])
        nc.gpsimd.dma_start(out=xk[2], in_=xd[:, 2])
        nc.scalar.dma_start(out=b1s, in_=b1t)

        for it in range(IT):
            w1t = w1pool.tile([128, T1, 2, 128], f8, name="w1t", tag="w1t")
            nc.sync.dma_start(out=w1t, in_=w1[:, it])
            ph = psum.tile([128, C], f32, name="ph", tag="pt")
            for t in range(T1):
                lhsT = w1t[:, t, :, :]
                for cc in range(4):
                    if t == 0:
                        rhs = xk0h[cc // 2][:, :, (cc % 2) * 256:
                                            (cc % 2) * 256 + 256]
                    else:
                        rhs = xk[t][:, :, cc * 256:(cc + 1) * 256]
                    # PSUM groups are bank-granular (2KB): within a bank
                    # (2 x 256-token chunks) only the first chunk starts the
                    # group and only the last stops it; the middle chunk's
                    # first write lands on pending-zero bytes
                    nc.tensor.matmul(
                        ph[:, cc * 256:(cc + 1) * 256],
                        lhsT=lhsT,
                        rhs=rhs,
                        start=(t == 0 and cc % 2 == 0),
                        stop=(t == T1 - 1 and cc % 2 == 1),
                        perf_mode=DR,
                    )
            nc.scalar.activation(hT[:, it // 2, it % 2, :], ph, act_fn,
                                 bias=b1s[:, it:it + 1], scale=1.0 / WS)
            # W2 chunks stream on the gpsimd queue, each pinned behind a
            # gelu (dummy copy -> WAW on the tile) so the scheduler can't
            # hoist them into the startup window where they'd starve the
            # critical w1 stream
            if it % 4 == 1 and it // 4 < T2 // 2:
                i = it // 4
                nc.vector.tensor_copy(
                    out=w2s[i][:, 0, 0, 0:4].bitcast(f32),
                    in_=hT[:, it // 2, it % 2, 0:4].bitcast(f32))
                nc.gpsimd.dma_start(out=w2s[i], in_=w2[:, i])
            # first two residual blocks prefetch at the phase-1 tail; the
            # rest chain off phase-2 progress below
            if it >= IT - 2:
                mtp = it - (IT - 2)
                nc.vector.tensor_copy(
                    out=xns[mtp][:, 0:1],
                    in_=hT[:, it // 2, it % 2, 0:4].bitcast(f32))
                nc.gpsimd.dma_start(out=xns[mtp], in_=xn[:, mtp])

        # ---- phase 2: y = hT.T @ W2s; z = y + 512*(x + b2); LayerNorm ----
        for mt in range(MT):
            # prefetch the residual block two iterations ahead, pinned
            # behind this iteration's (already-arrived) block
            if mt + 2 < MT:
                nc.vector.tensor_copy(out=xns[mt + 2][:, 0:1],
                                      in_=xns[mt][:, 0:1])
                nc.gpsimd.dma_start(out=xns[mt + 2], in_=xn[:, mt + 2])
            py = psum.tile([128, C], f32, name="py", tag="pt")
            for t in range(T2):
                lhsT = hT[:, t, :, mt * 128:(mt + 1) * 128]
                for hc in range(3):
                    # hc0+hc1 share PSUM bank 0, hc2 owns bank 1
                    nc.tensor.matmul(
                        py[:, hc * 256:(hc + 1) * 256],
                        lhsT=lhsT,
                        rhs=w2s[t // 2][:, t % 2, :,
                                        hc * 256:(hc + 1) * 256],
                        start=(t == 0 and hc != 1),
                        stop=(t == T2 - 1 and hc != 0),
                        perf_mode=DR,
                    )
            # residual add in place (DVE: the Pool engine cannot read PSUM)
            z = xns[mt]
            nc.vector.tensor_add(z, py[:, 0:H], z)
            stats = spool.tile([128, 3, 6], f32, name="stats", tag="stats")
            # g2 first: its half is ready as soon as the DVE add retires
            nc.vector.bn_stats(stats[:, 2], z[:, 512:768])
            nc.vector.bn_stats(stats[:, 0], z[:, 0:256])
            nc.vector.bn_stats(stats[:, 1], z[:, 256:512])
            mv = spool.tile([128, 2], f32, name="mv", tag="mv")
            nc.vector.bn_aggr(mv, stats)
            rstd = spool.tile([128, 1], f32, name="rstd", tag="rstd")
            nc.scalar.activation(rstd, mv[:, 1:2], AF.Sqrt, bias=epsT)
            nc.vector.reciprocal(out=rstd, in_=rstd)
            # normalize as z*rstd + (-mean*rstd) on the Scalar engine,
            # stores on the (idle) sync queue; halves overlap the tail
            nmr = spool.tile([128, 1], f32, name="nmr", tag="nmr")
            nc.vector.tensor_scalar(
                out=nmr, in0=mv[:, 0:1], scalar1=rstd, scalar2=-1.0,
                op0=ALU.mult, op1=ALU.mult)
            zo = zpool.tile([128, H], f32, name="zot", tag="zot")
            for h0 in (0, H // 2):
                sl = slice(h0, h0 + H // 2)
                nc.scalar.activation(zo[:, sl], z[:, sl], AF.Identity,
                                     bias=nmr, scale=rstd)
                nc.sync.dma_start(out=out[:, mt, sl], in_=zo[:, sl])

    nc.compile()
    return nc


def _get_nc(act_name="Gelu"):
    key = ("nc", act_name)
    if key not in _CACHE:
        _CACHE[key] = _build_nc(act_name)
    return _CACHE[key]


def _shard_inputs(x, task_ids, W1, b1, W2, b2):
    """Host-side dispatch: stable-sort tokens by expert id, chunk into E
    equal capacity-C blocks (exactly the reference's xs = x[order].reshape),
    then quantize/pack into the DoubleRow fp8 operand layouts."""
    import ml_dtypes

    f8 = ml_dtypes.float8_e4m3
    expert = (task_ids.astype(np.int64) % E).astype(np.int32)
    order = np.argsort(expert, kind="stable")
    xs = x[order]
    in_maps = []
    for e in range(E):
        xe = xs[e * C:(e + 1) * C]                       # [C, H]
        # xd[p, t, j, c] = fp8(xe[c, 256t + 128j + p])
        xq = np.ascontiguousarray(xe.T).astype(f8)
        xd = xq.reshape(T1, 2, 128, C).transpose(2, 0, 1, 3)
        # w1[p, it, t, j, m] = fp8(512 * W1[256t+128j+p, 128it+m])
        w1q = (W1[e] * WS).astype(f8)
        w1d = w1q.reshape(T1, 2, 128, IT, 128).transpose(2, 3, 0, 1, 4)
        # w2[p, tp, ts, j, h] = fp8(512 * W2[512tp+256ts+128j+p, h])
        w2q = (W2[e] * WS).astype(f8)
        w2d = w2q.reshape(T2 // 2, 2, 2, 128, H).transpose(3, 0, 1, 2, 4)
        b1t = b1[e].reshape(IT, 128).T
        xnv = ((xe + b2[e][None, :]) * WS).reshape(MT, 128, H).transpose(1, 0, 2)
        in_maps.append({
            "xd": np.ascontiguousarray(xd),
            "w1": np.ascontiguousarray(w1d),
            "w2": np.ascontiguousarray(w2d),
            "b1t": np.ascontiguousarray(b1t, dtype=np.float32),
            "xn": np.ascontiguousarray(xnv, dtype=np.float32),
        })
    return in_maps, order


def kernel(x, task_ids, W1, b1, W2, b2, gamma, beta):
    from concourse import bass_utils

    x = np.asarray(x, dtype=np.float32)
    task_ids = np.asarray(task_ids)
    W1 = np.asarray(W1, dtype=np.float32)
    b1 = np.asarray(b1, dtype=np.float32)
    W2 = np.asarray(W2, dtype=np.float32)
    b2 = np.asarray(b2, dtype=np.float32)
    gamma = np.asarray(gamma, dtype=np.float32)
    beta = np.asarray(beta, dtype=np.float32)

    in_maps, order = _shard_inputs(x, task_ids, W1, b1, W2, b2)
    nc = _get_nc()
    res = bass_utils.run_bass_kernel_spmd(nc, in_maps, core_ids=list(range(E)))
    _CACHE["last_results"] = res

    z = np.concatenate(
        [res.results[e]["out"].transpose(1, 0, 2).reshape(C, H) for e in range(E)],
        axis=0)
    # per-expert gamma/beta (identity for this problem's inputs; applied on
    # host only when nontrivial, matching the reference's z*gamma + beta)
    if not (np.all(gamma == 1.0) and np.all(beta == 0.0)):
        blk = np.repeat(np.arange(E), C)  # reference uses capacity blocks
        z = z * gamma[blk] + beta[blk]
    out = np.empty((N, H), dtype=np.float32)
    out[order] = z
    return out
